# revision 5
# baseline (speedup 1.0000x reference)
"""Trainium2 Bass kernel for nn_DA_conv (dense_cnn).

Model (per batch element b, channels c):
  kern = leaky(d @ kW1.T) @ kW2.T            -> per-(b,c) 3x3 depthwise filter
  dw   = depthwise_conv3x3(x, kern), pad=1   (cross-correlation)
  act  = leaky(dw)
  out  = conv1x1(act, convW) + convB + x * sigmoid-attention(d)

Sharding: data-parallel over batch B=16 across 8 cores (2 images/core).
Per-core layout: 128 SBUF partitions = (2 images x 64 channels); spatial
plane stored flat with 1-pixel zero padding: rows of 130 floats.

Engine split per 11-row band:
  - PE  : 9 depthwise taps as diag-matrix matmuls (bf16) accumulated in
          PSUM + 1x1 conv as block-diag matmul + bias via K=1 ones-matmul
  - ACT : fp32->bf16 cast of the band, leaky (Prelu alpha=0.1) PSUM->SBUF
  - DVE : final combine (x*att + psum) evacuation; plus a few whole bands
          computed on DVE via scalar_tensor_tensor taps to offload PE
The tiny d-MLPs (kern, att) run on-device with fp32 matmuls; the
per-(b,c)-filter rearranges go through small DRAM scratch roundtrips.
"""
import numpy as np
import ml_dtypes

import concourse.bacc as bacc
import concourse.bass as bass
import concourse.mybir as mybir
import concourse.tile as tile
from concourse.bass_utils import run_bass_kernel_spmd
from concourse.masks import make_identity

F32 = mybir.dt.float32
BF16 = mybir.dt.bfloat16
AF = mybir.ActivationFunctionType
ALU = mybir.AluOpType

B, C, H, W = 16, 64, 128, 128
NCORES = 8
BL = B // NCORES          # images per core (2)
P = BL * C                # partitions used (128)
WP = W + 2                # padded row length (130)
NEG = 0.1                 # leaky slope

BAND = 11                 # interior rows per band
DVE_BANDS = (5, 11)       # bands whose depthwise runs on DVE instead of PE

_CACHE = {}


def _bands():
    out = []
    p0 = 0
    while p0 < H:
        nr = min(BAND, H - p0)
        out.append((p0, nr))
        p0 += nr
    return out


def _chunks(span):
    return [(cs, min(512, span - cs)) for cs in range(0, span, 512)]


def _build():
    nc = bacc.Bacc("TRN2", target_bir_lowering=False, debug=False)

    x_d = nc.dram_tensor("x", [BL, C, H, W], F32, kind="ExternalInput")
    dT_d = nc.dram_tensor("dT", [C, BL], F32, kind="ExternalInput")
    kW1T_d = nc.dram_tensor("kW1T", [C, C], F32, kind="ExternalInput")
    kW2T_d = nc.dram_tensor("kW2T", [C, C * 9], F32, kind="ExternalInput")
    caW1T_d = nc.dram_tensor("caW1T", [C, 8], F32, kind="ExternalInput")
    caW2T_d = nc.dram_tensor("caW2T", [8, C], F32, kind="ExternalInput")
    cwbd_d = nc.dram_tensor("convWbd16", [P, P], BF16, kind="ExternalInput")
    cb2_d = nc.dram_tensor("convB2_16", [1, P], BF16, kind="ExternalInput")
    out_d = nc.dram_tensor("out", [BL, C, H, W], F32, kind="ExternalOutput")

    with tile.TileContext(nc) as tc:
        with (
            tc.tile_pool(name="consts", bufs=1) as consts,
            tc.tile_pool(name="xb", bufs=3) as xbp,
            tc.tile_pool(name="xb16", bufs=3) as xb16p,
            tc.tile_pool(name="actb", bufs=3) as actbp,
            tc.tile_pool(name="accb", bufs=2) as accbp,
            tc.tile_pool(name="outb", bufs=3) as outbp,
            tc.tile_pool(name="psA", bufs=2, space="PSUM") as psA,
            tc.tile_pool(name="psB", bufs=2, space="PSUM") as psB,
            tc.tile_pool(name="dram", bufs=1, space="DRAM") as dram,
        ):
            # ---- load weights/inputs that persist ----
            dT = consts.tile([C, BL], F32)
            kW1T = consts.tile([C, C], F32)
            kW2T = consts.tile([C, C * 9], F32)
            caW1T = consts.tile([C, 8], F32)
            caW2T = consts.tile([8, C], F32)
            cwbd = consts.tile([P, P], BF16)
            cb2 = consts.tile([1, P], BF16)
            nc.sync.dma_start(out=dT, in_=dT_d.ap())
            nc.sync.dma_start(out=kW1T, in_=kW1T_d.ap())
            nc.sync.dma_start(out=kW2T, in_=kW2T_d.ap())
            nc.sync.dma_start(out=caW1T, in_=caW1T_d.ap())
            nc.sync.dma_start(out=caW2T, in_=caW2T_d.ap())
            nc.sync.dma_start(out=cwbd, in_=cwbd_d.ap())
            nc.sync.dma_start(out=cb2, in_=cb2_d.ap())

            ones16 = consts.tile([1, 512], BF16)
            nc.gpsimd.memset(ones16, 1.0)
            ident = consts.tile([P, P], F32)
            make_identity(nc, ident)

            # ---- kern MLP: kern = leaky(d @ kW1.T) @ kW2.T ----
            h1p = psB.tile([C, BL], F32, tag="B")
            nc.tensor.matmul(h1p, kW1T, dT, start=True, stop=True)
            h1 = consts.tile([C, BL], F32, tag="h1")
            nc.scalar.activation(h1, h1p, AF.Prelu, alpha=NEG)

            kernp = psB.tile([P, 10], F32, tag="B")
            for j in range(5):
                m = min(128, C * 9 - 128 * j)
                nc.tensor.matmul(kernp[0:m, 2 * j:2 * j + 2],
                                 kW2T[:, 128 * j:128 * j + m], h1,
                                 start=True, stop=True)
            kernf = consts.tile([P, 10], F32, tag="kernf")
            nc.scalar.copy(kernf, kernp)

            # roundtrip through DRAM to re-lay kern as [(b,c), tap]
            skern = dram.tile([C * 9 * BL], F32)
            sk_t = skern.tensor
            # flat scratch address = j*2 + b with j = jc*128 + p (j < 576)
            nc.sync.dma_start(
                out=bass.AP(tensor=sk_t, offset=skern.offset,
                            ap=[[2, 128], [256, 4], [1, 2]]),
                in_=kernf[:, 0:8].rearrange("p (j b) -> p j b", b=2))
            nc.sync.dma_start(
                out=bass.AP(tensor=sk_t, offset=skern.offset + 1024,
                            ap=[[2, 64], [1, 2]]),
                in_=kernf[0:64, 8:10])
            kern_pp = consts.tile([P, 9], F32, tag="kern_pp")
            for b in range(2):
                nc.sync.dma_start(
                    out=kern_pp[64 * b:64 * (b + 1), :],
                    in_=bass.AP(tensor=sk_t, offset=skern.offset + b,
                                ap=[[18, 64], [2, 9]]))

            # diag tap matrices (bf16): diag16[:, t*128:(t+1)*128] = I * kern_t
            diag16 = consts.tile([P, 9 * P], BF16, tag="diag16")
            for t in range(9):
                nc.vector.tensor_scalar(diag16[:, P * t:P * (t + 1)], ident,
                                        kern_pp[:, t:t + 1], None, ALU.mult)

            # ---- attention MLP: att = sigmoid(leaky(d @ caW1.T) @ caW2.T) ----
            a1p = psB.tile([8, BL], F32, tag="B")
            nc.tensor.matmul(a1p, caW1T, dT, start=True, stop=True)
            a1 = consts.tile([8, BL], F32, tag="a1")
            nc.scalar.activation(a1, a1p, AF.Prelu, alpha=NEG)
            attp = psB.tile([C, BL], F32, tag="B")
            nc.tensor.matmul(attp, caW2T, a1, start=True, stop=True)
            atts = consts.tile([C, BL], F32, tag="atts")
            nc.scalar.activation(atts, attp, AF.Sigmoid)

            satt = dram.tile([P], F32)
            nc.sync.dma_start(
                out=bass.AP(tensor=satt.tensor, offset=satt.offset,
                            ap=[[1, 64], [64, 2]]),
                in_=atts)
            att_pp = consts.tile([P, 1], F32, tag="att_pp")
            nc.sync.dma_start(
                out=att_pp,
                in_=bass.AP(tensor=satt.tensor, offset=satt.offset,
                            ap=[[1, 128], [0, 1]]))

            # ---- main banded loop ----
            for bi, (p0, nr) in enumerate(_bands()):
                R = nr + 2                    # padded rows in this band's tile
                span = (nr - 1) * WP + W      # flat interior output span
                base = WP + 1                 # offset of first interior output

                xb = xbp.tile([P, R * WP], F32, tag="xb")
                xbv = xb.rearrange("p (r w) -> p r w", w=WP)
                # zero the left/right padding columns
                nc.scalar.memzero(xbv[:, :, 0:1])
                nc.scalar.memzero(xbv[:, :, W + 1:W + 2])
                # zero top/bottom padding rows (first/last band only)
                r_lo = max(0, 1 - p0)
                r_hi = min(R, 129 - p0)
                if r_lo > 0:
                    nc.scalar.memzero(xbv[:, 0:r_lo, 1:W + 1])
                if r_hi < R:
                    nc.scalar.memzero(xbv[:, r_hi:R, 1:W + 1])
                nc.sync.dma_start(
                    out=xbv[:, r_lo:r_hi, 1:W + 1],
                    in_=x_d.ap().rearrange("b c h w -> (b c) h w")
                    [:, p0 + r_lo - 1:p0 + r_hi - 1, :])

                actb = actbp.tile([P, span], BF16, tag="actb")
                if bi not in DVE_BANDS:
                    # PE depthwise: bf16 cast + 9 diag matmuls per window
                    xb16 = xb16p.tile([P, R * WP], BF16, tag="xb16")
                    nc.scalar.copy(xb16, xb)
                    pa = psA.tile([P, span], F32, tag="A")
                    for (cs, wn) in _chunks(span):
                        for t in range(9):
                            ky, kx = t // 3, t % 3
                            off = base + (ky - 1) * WP + (kx - 1) + cs
                            nc.tensor.matmul(
                                pa[:, cs:cs + wn],
                                diag16[:, P * t:P * (t + 1)],
                                xb16[:, off:off + wn],
                                start=(t == 0), stop=(t == 8))
                    nc.scalar.activation(actb, pa, AF.Prelu, alpha=NEG)
                else:
                    # DVE depthwise: scalar_tensor_tensor tap chain
                    acc = accbp.tile([P, span], F32, tag="acc")
                    for t in range(9):
                        ky, kx = t // 3, t % 3
                        off = base + (ky - 1) * WP + (kx - 1)
                        src = xb[:, off:off + span]
                        if t == 0:
                            nc.vector.tensor_scalar(
                                acc, src, kern_pp[:, 0:1], None, ALU.mult)
                        else:
                            nc.vector.scalar_tensor_tensor(
                                acc, src, kern_pp[:, t:t + 1], acc,
                                op0=ALU.mult, op1=ALU.add)
                    nc.vector.scalar_tensor_tensor(
                        actb, acc, NEG, acc, op0=ALU.mult, op1=ALU.max)

                # 1x1 conv + bias + x*att residual, chunked by PSUM bank
                outb = outbp.tile([P, nr * WP], F32, tag="outb")
                for (cs, wn) in _chunks(span):
                    pb = psB.tile([P, 512], F32, tag="B")
                    nc.tensor.matmul(pb[:, 0:wn], cwbd, actb[:, cs:cs + wn],
                                     start=True, stop=False)
                    nc.tensor.matmul(pb[:, 0:wn], cb2, ones16[:, 0:wn],
                                     start=False, stop=True)
                    nc.vector.scalar_tensor_tensor(
                        outb[:, cs:cs + wn], xb[:, base + cs:base + cs + wn],
                        att_pp[:, 0:1], pb[:, 0:wn],
                        op0=ALU.mult, op1=ALU.add)

                nc.sync.dma_start(
                    out=out_d.ap().rearrange("b c h w -> (b c) h w")
                    [:, p0:p0 + nr, :],
                    in_=outb.rearrange("p (r w) -> p r w", w=WP)[:, :, 0:W])

    nc.compile()
    return nc


def _prep_shared(kW1, kW2, convW, convB, caW1, caW2):
    cwbd = np.zeros((P, P), np.float32)
    cwbd[0:C, 0:C] = convW.T
    cwbd[C:P, C:P] = convW.T
    return {
        "kW1T": np.ascontiguousarray(kW1.T),
        "kW2T": np.ascontiguousarray(kW2.T),
        "caW1T": np.ascontiguousarray(caW1.T),
        "caW2T": np.ascontiguousarray(caW2.T),
        "convWbd16": cwbd.astype(ml_dtypes.bfloat16),
        "convB2_16": np.tile(convB, 2)[None, :].astype(ml_dtypes.bfloat16),
    }


def kernel(x, d, kW1, kW2, convW, convB, caW1, caW2, _trace=False):
    x = np.asarray(x, np.float32)
    d = np.asarray(d, np.float32)
    if "nc" not in _CACHE:
        _CACHE["nc"] = _build()
    nc = _CACHE["nc"]

    shared = _prep_shared(np.asarray(kW1, np.float32),
                          np.asarray(kW2, np.float32),
                          np.asarray(convW, np.float32),
                          np.asarray(convB, np.float32),
                          np.asarray(caW1, np.float32),
                          np.asarray(caW2, np.float32))
    in_maps = []
    for c in range(NCORES):
        sl = slice(c * BL, (c + 1) * BL)
        m = dict(shared)
        m["x"] = np.ascontiguousarray(x[sl])
        m["dT"] = np.ascontiguousarray(d[sl].T)
        in_maps.append(m)

    res = run_bass_kernel_spmd(nc, in_maps, core_ids=list(range(NCORES)),
                               trace=_trace)
    out = np.concatenate([r["out"] for r in res.results], axis=0)
    if _trace:
        return out, res
    return out
